# revision 6
# baseline (speedup 1.0000x reference)
"""Trainium2 Bass kernel for nn_HAN_47321949667634 (RGCN + KGE scoring + HAN head).

Strategy (8 NeuronCores, SPMD, two launches, no collectives):
  Launch 1: entity table row-sharded 12500 rows/core. Each core aggregates the
    KG edges whose dst lands in its shard (edges gathered from a replicated E
    via indirect DMA, one-hot scatter matmul per 128-node window), applies the
    per-relation transforms W_r and Wroot with the output kept transposed
    [feat, nodes], and emits tanh(...) = E_new^T for its shard.
  Host: re-slices E_new between launches (stand-in for the all-to-all).
  Launch 2: pred = x @ E_new^T column-parallel (each core scores its shard's
    12500 columns); the small metapath RGCNs (only the first NREG=2000 rows
    survive), semantic attention and the E_reg head run replicated on every
    core with host-pregathered edge features; host keeps core 0's copy.

All integer index manipulation (edge partition/sort, chunk planning, padding)
happens on host at program-build time; all floating-point math and bulk
gathers run on device.
"""
import numpy as np

import concourse.tile as tile
from concourse import bass, bacc, bass_utils, mybir
from concourse.bass import IndirectOffsetOnAxis

dt = mybir.dt
f32 = dt.float32
u32 = dt.uint32

NE = 100000
NR = 10
EDIM = 128
HID = 256
NREG = 2000
NMP_REL = 5
B = 1024
ATT = 128

NCORE = 8
SH = NE // NCORE          # 12500 rows per core
P = 128
NW = (SH + P - 1) // P    # 98 windows (last = 84 nodes)
NW2 = (NREG + P - 1) // P  # 16 windows over region rows (last = 80)
REGPAD = NW2 * P           # 2048

_cache = {}


# --------------------------------------------------------------------------
# planning (host, integer-only)
# --------------------------------------------------------------------------

def _plan_k1(src, dst, et):
    """Uniform-across-cores chunk plan for the main RGCN aggregation."""
    core = dst // SH
    nl = dst % SH
    w = nl // P
    dl = nl % P
    gid = w * NR + et                      # group id within a core
    ngrp = NW * NR

    counts = np.zeros((NCORE, ngrp), np.int64)
    np.add.at(counts, (core, gid), 1)
    maxc = counts.max(axis=0)
    K = (maxc + P - 1) // P                # chunks per group (0 if empty)
    col_base = np.zeros(ngrp + 1, np.int64)
    col_base[1:] = np.cumsum(K)
    nc1 = int(col_base[-1])

    plan = []
    for wi in range(NW):
        groups = []
        for r in range(NR):
            g = wi * NR + r
            if maxc[g] == 0:
                continue
            chunks = [(int(col_base[g] + j), int(min(P, maxc[g] - j * P)))
                      for j in range(int(K[g]))]
            groups.append((r, chunks))
        plan.append(groups)

    srcix = np.zeros((NCORE, P, nc1), np.uint32)
    dstl = np.full((NCORE, P, nc1), -1.0, np.float32)
    for c in range(NCORE):
        m = core == c
        g = gid[m]
        s = src[m]
        d = dl[m]
        order = np.argsort(g, kind="stable")
        g, s, d = g[order], s[order], d[order]
        starts = np.searchsorted(g, np.arange(ngrp))
        pos = np.arange(len(g)) - starts[g]
        colv = col_base[g] + pos // P
        rowv = pos % P
        srcix[c, rowv, colv] = s
        dstl[c, rowv, colv] = d
    return plan, nc1, srcix, dstl


def _plan_k2(mps):
    """Chunk plan for both metapath RGCNs (dst < NREG only; same on all cores).

    mps: list of (src, dst, et, eids) int arrays.
    Returns plan2 (per (m, w): [(r, [(col, row_off, ne)])]), nc2, dstl [P, nc2],
    gidx (global E_new row index per gathered edge row, chunk-ordered),
    root_gidx [2, NREG].
    """
    plan2 = []
    dstl_cols = []
    gidx_parts = []
    col = 0
    row_off = 0
    root_gidx = []
    for m, (src, dst, et, eids) in enumerate(mps):
        keep = dst < NREG
        src, dst, et = src[keep], dst[keep], et[keep]
        w = dst // P
        dl = dst % P
        gid = w * NMP_REL + et
        order = np.argsort(gid, kind="stable")
        gid, src_s, dl_s = gid[order], src[order], dl[order]
        ngrp = NW2 * NMP_REL
        starts = np.searchsorted(gid, np.arange(ngrp + 1))
        mplan = []
        for wi in range(NW2):
            groups = []
            for r in range(NMP_REL):
                g = wi * NMP_REL + r
                a, b = int(starts[g]), int(starts[g + 1])
                if a == b:
                    continue
                chunks = []
                for j in range(a, b, P):
                    ne = min(P, b - j)
                    cc = np.full((P,), -1.0, np.float32)
                    cc[:ne] = dl_s[j:j + ne]
                    dstl_cols.append(cc)
                    gidx_parts.append(eids[src_s[j:j + ne]])
                    chunks.append((col, row_off, ne))
                    col += 1
                    row_off += ne
                groups.append((r, chunks))
            mplan.append(groups)
        plan2.append(mplan)
        root_gidx.append(eids[:NREG])
    nc2 = col
    dstl = np.stack(dstl_cols, axis=1).astype(np.float32) if nc2 else \
        np.zeros((P, 1), np.float32)
    gidx = np.concatenate(gidx_parts) if gidx_parts else np.zeros(0, np.int64)
    return plan2, max(nc2, 1), dstl, gidx, np.stack(root_gidx)


# --------------------------------------------------------------------------
# program builders
# --------------------------------------------------------------------------

def _iota_tile():
    return np.broadcast_to(np.arange(P, dtype=np.float32)[None, :], (P, P)).copy()


def _build_k1(plan, nc1):
    nc = bacc.Bacc("TRN2", target_bir_lowering=False, debug=False,
                   num_devices=NCORE)
    e_full = nc.dram_tensor("e_full", [NE, EDIM], f32, kind="ExternalInput")
    e_t = nc.dram_tensor("e_t", [EDIM, SH], f32, kind="ExternalInput")
    w_all = nc.dram_tensor("w_all", [EDIM, NR * EDIM], f32, kind="ExternalInput")
    wroot = nc.dram_tensor("wroot", [EDIM, EDIM], f32, kind="ExternalInput")
    bvec = nc.dram_tensor("bvec", [EDIM, 1], f32, kind="ExternalInput")
    iota = nc.dram_tensor("iota", [P, P], f32, kind="ExternalInput")
    srcidx = nc.dram_tensor("srcidx", [P, nc1], u32, kind="ExternalInput")
    dstl = nc.dram_tensor("dstl", [P, nc1], f32, kind="ExternalInput")
    out = nc.dram_tensor("e_newt_out", [EDIM, SH], f32, kind="ExternalOutput")

    with tile.TileContext(nc) as tc:
        with tc.tile_pool(name="const", bufs=1) as cp, \
             tc.tile_pool(name="work", bufs=4) as wp, \
             tc.tile_pool(name="psum", bufs=2, space="PSUM") as pp:
            et_sb = cp.tile([EDIM, SH], f32, tag="et")
            nc.sync.dma_start(out=et_sb[:], in_=e_t.ap()[:, :])
            w_sb = cp.tile([EDIM, NR * EDIM], f32, tag="w")
            nc.sync.dma_start(out=w_sb[:], in_=w_all.ap()[:, :])
            wr_sb = cp.tile([EDIM, EDIM], f32, tag="wr")
            nc.sync.dma_start(out=wr_sb[:], in_=wroot.ap()[:, :])
            b_sb = cp.tile([EDIM, 1], f32, tag="b")
            nc.sync.dma_start(out=b_sb[:], in_=bvec.ap()[:, :])
            io_sb = cp.tile([P, P], f32, tag="io")
            nc.sync.dma_start(out=io_sb[:], in_=iota.ap()[:, :])
            si_sb = cp.tile([P, nc1], u32, tag="si")
            nc.sync.dma_start(out=si_sb[:], in_=srcidx.ap()[:, :])
            dl_sb = cp.tile([P, nc1], f32, tag="dl")
            nc.sync.dma_start(out=dl_sb[:], in_=dstl.ap()[:, :])
            ent_sb = cp.tile([EDIM, SH], f32, tag="ent")

            for wi in range(NW):
                w0 = wi * P
                wn = min(P, SH - w0)
                groups = plan[wi]
                po = pp.tile([P, P], f32, tag="po")
                nc.tensor.matmul(po[:, :wn], lhsT=wr_sb[:],
                                 rhs=et_sb[:, w0:w0 + wn],
                                 start=True, stop=(len(groups) == 0))
                for gi, (r, chunks) in enumerate(groups):
                    pa = pp.tile([P, P], f32, tag="pa")
                    for j, (col, ne) in enumerate(chunks):
                        xg = wp.tile([P, EDIM], f32, tag="xg")
                        nc.gpsimd.indirect_dma_start(
                            out=xg[:ne], out_offset=None,
                            in_=e_full.ap()[:, :],
                            in_offset=IndirectOffsetOnAxis(
                                ap=si_sb[:ne, col:col + 1], axis=0))
                        pm = wp.tile([P, P], f32, tag="pm")
                        nc.vector.tensor_scalar(
                            out=pm[:ne, :wn], in0=io_sb[:ne, :wn],
                            scalar1=dl_sb[:ne, col:col + 1], scalar2=None,
                            op0=mybir.AluOpType.is_equal)
                        nc.tensor.matmul(pa[:, :wn], lhsT=xg[:ne, :],
                                         rhs=pm[:ne, :wn],
                                         start=(j == 0),
                                         stop=(j == len(chunks) - 1))
                    ag = wp.tile([P, P], f32, tag="ag")
                    nc.vector.tensor_copy(ag[:, :wn], pa[:, :wn])
                    nc.tensor.matmul(po[:, :wn],
                                     lhsT=w_sb[:, r * EDIM:(r + 1) * EDIM],
                                     rhs=ag[:, :wn],
                                     start=False, stop=(gi == len(groups) - 1))
                nc.scalar.activation(out=ent_sb[:, w0:w0 + wn],
                                     in_=po[:, :wn],
                                     func=mybir.ActivationFunctionType.Tanh,
                                     bias=b_sb[:, 0:1], scale=1.0)
            nc.sync.dma_start(out=out.ap()[:, :], in_=ent_sb[:])
    nc.compile()
    return nc


def _build_k2(plan2, nc2, n_gx):
    nc = bacc.Bacc("TRN2", target_bir_lowering=False, debug=False,
                   num_devices=NCORE)
    ent = nc.dram_tensor("ent", [EDIM, SH], f32, kind="ExternalInput")
    xht = nc.dram_tensor("xht", [EDIM, B], f32, kind="ExternalInput")
    rht = nc.dram_tensor("rht", [EDIM, B], f32, kind="ExternalInput")
    mpx = nc.dram_tensor("mpx", [max(n_gx, 1), EDIM], f32, kind="ExternalInput")
    mpdstl = nc.dram_tensor("mpdstl", [P, nc2], f32, kind="ExternalInput")
    mproott = nc.dram_tensor("mproott", [EDIM, 2 * REGPAD], f32,
                             kind="ExternalInput")
    mpw = nc.dram_tensor("mpw", [EDIM, 2 * NMP_REL * HID], f32,
                         kind="ExternalInput")
    mpwroot = nc.dram_tensor("mpwroot", [EDIM, 2 * HID], f32,
                             kind="ExternalInput")
    mpb = nc.dram_tensor("mpb", [P, 4], f32, kind="ExternalInput")
    saw1 = nc.dram_tensor("saw1", [P, 2 * ATT], f32, kind="ExternalInput")
    sab1 = nc.dram_tensor("sab1", [P, 1], f32, kind="ExternalInput")
    saw2 = nc.dram_tensor("saw2", [P, 1], f32, kind="ExternalInput")
    predw = nc.dram_tensor("predw", [P, 2 * EDIM], f32, kind="ExternalInput")
    predb = nc.dram_tensor("predb", [P, 1], f32, kind="ExternalInput")
    ones_row = nc.dram_tensor("ones_row", [1, P], f32, kind="ExternalInput")
    iota = nc.dram_tensor("iota", [P, P], f32, kind="ExternalInput")

    pred_out = nc.dram_tensor("pred_out", [B, SH], f32, kind="ExternalOutput")
    ereg_out = nc.dram_tensor("ereg_out", [EDIM, REGPAD], f32,
                              kind="ExternalOutput")

    NSTRIP = 512

    with tile.TileContext(nc) as tc:
        with tc.tile_pool(name="const", bufs=1) as cp, \
             tc.tile_pool(name="work", bufs=4) as wp, \
             tc.tile_pool(name="stage", bufs=3) as sp, \
             tc.tile_pool(name="psum", bufs=2, space="PSUM") as pp, \
             tc.tile_pool(name="psum1", bufs=1, space="PSUM") as pp1:
            ent_sb = cp.tile([EDIM, SH], f32, tag="ent")
            nc.sync.dma_start(out=ent_sb[:], in_=ent.ap()[:, :])
            xht_sb = cp.tile([EDIM, B], f32, tag="xht")
            nc.sync.dma_start(out=xht_sb[:], in_=xht.ap()[:, :])
            rht_sb = cp.tile([EDIM, B], f32, tag="rht")
            nc.sync.dma_start(out=rht_sb[:], in_=rht.ap()[:, :])
            dl_sb = cp.tile([P, nc2], f32, tag="dl")
            nc.sync.dma_start(out=dl_sb[:], in_=mpdstl.ap()[:, :])
            rt_sb = cp.tile([EDIM, 2 * REGPAD], f32, tag="rt")
            nc.sync.dma_start(out=rt_sb[:], in_=mproott.ap()[:, :])
            w_sb = cp.tile([EDIM, 2 * NMP_REL * HID], f32, tag="w")
            nc.sync.dma_start(out=w_sb[:], in_=mpw.ap()[:, :])
            wr_sb = cp.tile([EDIM, 2 * HID], f32, tag="wr")
            nc.sync.dma_start(out=wr_sb[:], in_=mpwroot.ap()[:, :])
            mb_sb = cp.tile([P, 4], f32, tag="mb")
            nc.sync.dma_start(out=mb_sb[:], in_=mpb.ap()[:, :])
            sw1_sb = cp.tile([P, 2 * ATT], f32, tag="sw1")
            nc.sync.dma_start(out=sw1_sb[:], in_=saw1.ap()[:, :])
            sb1_sb = cp.tile([P, 1], f32, tag="sb1")
            nc.sync.dma_start(out=sb1_sb[:], in_=sab1.ap()[:, :])
            sw2_sb = cp.tile([P, 1], f32, tag="sw2")
            nc.sync.dma_start(out=sw2_sb[:], in_=saw2.ap()[:, :])
            pw_sb = cp.tile([P, 2 * EDIM], f32, tag="pw")
            nc.sync.dma_start(out=pw_sb[:], in_=predw.ap()[:, :])
            pb_sb = cp.tile([P, 1], f32, tag="pb")
            nc.sync.dma_start(out=pb_sb[:], in_=predb.ap()[:, :])
            on_sb = cp.tile([1, P], f32, tag="on")
            nc.sync.dma_start(out=on_sb[:], in_=ones_row.ap()[:, :])
            io_sb = cp.tile([P, P], f32, tag="io")
            nc.sync.dma_start(out=io_sb[:], in_=iota.ap()[:, :])

            # ---------------- pred = x @ E_new^T (column-parallel) ----------
            xt_sb = cp.tile([EDIM, B], f32, tag="xt")
            nc.vector.tensor_tensor(out=xt_sb[:], in0=xht_sb[:],
                                    in1=rht_sb[:],
                                    op=mybir.AluOpType.mult)
            for bt in range(B // P):
                for s0 in range(0, SH, NSTRIP):
                    sw = min(NSTRIP, SH - s0)
                    pj = pp.tile([P, NSTRIP], f32, tag="pj")
                    nc.tensor.matmul(pj[:, :sw],
                                     lhsT=xt_sb[:, bt * P:(bt + 1) * P],
                                     rhs=ent_sb[:, s0:s0 + sw],
                                     start=True, stop=True)
                    st = sp.tile([P, NSTRIP], f32, tag="st")
                    nc.vector.tensor_copy(st[:, :sw], pj[:, :sw])
                    nc.sync.dma_start(
                        out=pred_out.ap()[bt * P:(bt + 1) * P, s0:s0 + sw],
                        in_=st[:, :sw])

            # ---------------- metapath RGCNs (replicated) -------------------
            sems = []   # [m][half] -> [128, REGPAD] tiles
            for m in range(2):
                lo = cp.tile([P, REGPAD], f32, tag=f"sem{m}lo")
                hi = cp.tile([P, REGPAD], f32, tag=f"sem{m}hi")
                nc.vector.memset(lo[:, NREG:REGPAD], 0.0)
                nc.vector.memset(hi[:, NREG:REGPAD], 0.0)
                sems.append([lo, hi])
                for wi in range(NW2):
                    w0 = wi * P
                    wn = min(P, NREG - w0)
                    groups = plan2[m][wi]
                    pol = pp1.tile([P, P], f32, tag="pol")
                    poh = pp1.tile([P, P], f32, tag="poh")
                    rtcol = m * REGPAD + w0
                    nc.tensor.matmul(pol[:, :wn],
                                     lhsT=wr_sb[:, (2 * m + 0) * P:(2 * m + 0) * P + P],
                                     rhs=rt_sb[:, rtcol:rtcol + wn],
                                     start=True, stop=(len(groups) == 0))
                    nc.tensor.matmul(poh[:, :wn],
                                     lhsT=wr_sb[:, (2 * m + 1) * P:(2 * m + 1) * P + P],
                                     rhs=rt_sb[:, rtcol:rtcol + wn],
                                     start=True, stop=(len(groups) == 0))
                    for gi, (r, chunks) in enumerate(groups):
                        pa = pp.tile([P, P], f32, tag="pa")
                        for j, (col, roff, ne) in enumerate(chunks):
                            xg = wp.tile([P, EDIM], f32, tag="xg")
                            nc.sync.dma_start(out=xg[:ne],
                                              in_=mpx.ap()[roff:roff + ne, :])
                            pm = wp.tile([P, P], f32, tag="pm")
                            nc.vector.tensor_scalar(
                                out=pm[:ne, :wn], in0=io_sb[:ne, :wn],
                                scalar1=dl_sb[:ne, col:col + 1], scalar2=None,
                                op0=mybir.AluOpType.is_equal)
                            nc.tensor.matmul(pa[:, :wn], lhsT=xg[:ne, :],
                                             rhs=pm[:ne, :wn],
                                             start=(j == 0),
                                             stop=(j == len(chunks) - 1))
                        ag = wp.tile([P, P], f32, tag="ag")
                        nc.vector.tensor_copy(ag[:, :wn], pa[:, :wn])
                        last = gi == len(groups) - 1
                        wcol = (m * NMP_REL + r) * HID
                        nc.tensor.matmul(pol[:, :wn],
                                         lhsT=w_sb[:, wcol:wcol + P],
                                         rhs=ag[:, :wn],
                                         start=False, stop=last)
                        nc.tensor.matmul(poh[:, :wn],
                                         lhsT=w_sb[:, wcol + P:wcol + 2 * P],
                                         rhs=ag[:, :wn],
                                         start=False, stop=last)
                    nc.scalar.activation(
                        out=lo[:, w0:w0 + wn], in_=pol[:, :wn],
                        func=mybir.ActivationFunctionType.Relu,
                        bias=mb_sb[:, 2 * m:2 * m + 1], scale=1.0)
                    nc.scalar.activation(
                        out=hi[:, w0:w0 + wn], in_=poh[:, :wn],
                        func=mybir.ActivationFunctionType.Relu,
                        bias=mb_sb[:, 2 * m + 1:2 * m + 2], scale=1.0)

            # ---------------- semantic attention ----------------------------
            wvals = []
            for m in range(2):
                wv = cp.tile([1, REGPAD], f32, tag=f"wv{m}")
                for s0 in range(0, REGPAD, NSTRIP):
                    pz = pp1.tile([P, NSTRIP], f32, tag="misc")
                    nc.tensor.matmul(pz[:], lhsT=sw1_sb[:, 0:ATT],
                                     rhs=sems[m][0][:, s0:s0 + NSTRIP],
                                     start=True, stop=False)
                    nc.tensor.matmul(pz[:], lhsT=sw1_sb[:, ATT:2 * ATT],
                                     rhs=sems[m][1][:, s0:s0 + NSTRIP],
                                     start=False, stop=True)
                    tz = wp.tile([P, NSTRIP], f32, tag="tz")
                    nc.scalar.activation(
                        out=tz[:], in_=pz[:],
                        func=mybir.ActivationFunctionType.Tanh,
                        bias=sb1_sb[:, 0:1], scale=1.0)
                    pw_ = pp1.tile([1, NSTRIP], f32, tag="misc")
                    nc.tensor.matmul(pw_[:], lhsT=sw2_sb[:, 0:1], rhs=tz[:],
                                     start=True, stop=True)
                    nc.vector.tensor_copy(wv[:, s0:s0 + NSTRIP], pw_[:])
                wvals.append(wv)

            b2 = cp.tile([1, 2], f32, tag="b2")
            for m in range(2):
                nc.vector.tensor_reduce(out=b2[0:1, m:m + 1],
                                        in_=wvals[m][0:1, 0:NREG],
                                        axis=mybir.AxisListType.X,
                                        op=mybir.AluOpType.add)
            nc.vector.tensor_scalar(out=b2[:], in0=b2[:],
                                    scalar1=1.0 / NREG, scalar2=None,
                                    op0=mybir.AluOpType.mult)
            mx = cp.tile([1, 1], f32, tag="mx")
            nc.vector.tensor_reduce(out=mx[:], in_=b2[:],
                                    axis=mybir.AxisListType.X,
                                    op=mybir.AluOpType.max)
            nc.vector.tensor_scalar(out=b2[:], in0=b2[:], scalar1=mx[0:1, 0:1],
                                    scalar2=None,
                                    op0=mybir.AluOpType.subtract)
            nc.scalar.activation(out=b2[:], in_=b2[:],
                                 func=mybir.ActivationFunctionType.Exp)
            sm = cp.tile([1, 1], f32, tag="sm")
            nc.vector.tensor_reduce(out=sm[:], in_=b2[:],
                                    axis=mybir.AxisListType.X,
                                    op=mybir.AluOpType.add)
            si = cp.tile([1, 1], f32, tag="si")
            nc.vector.reciprocal(si[:], sm[:])
            nc.vector.tensor_scalar(out=b2[:], in0=b2[:], scalar1=si[0:1, 0:1],
                                    scalar2=None, op0=mybir.AluOpType.mult)
            pbb = pp1.tile([P, 2], f32, tag="misc")
            nc.tensor.matmul(pbb[:], lhsT=on_sb[0:1, :], rhs=b2[0:1, :],
                             start=True, stop=True)
            bb = cp.tile([P, 2], f32, tag="bb")
            nc.vector.tensor_copy(bb[:], pbb[:])

            # h^T = beta0 * sems0 + beta1 * sems1, per half
            ht = []
            for half in range(2):
                h_sb = cp.tile([P, REGPAD], f32, tag=f"ht{half}")
                tmp = cp.tile([P, REGPAD], f32, tag=f"httmp{half}")
                nc.vector.tensor_scalar(out=h_sb[:], in0=sems[0][half][:],
                                        scalar1=bb[:, 0:1], scalar2=None,
                                        op0=mybir.AluOpType.mult)
                nc.vector.tensor_scalar(out=tmp[:], in0=sems[1][half][:],
                                        scalar1=bb[:, 1:2], scalar2=None,
                                        op0=mybir.AluOpType.mult)
                nc.vector.tensor_tensor(out=h_sb[:], in0=h_sb[:], in1=tmp[:],
                                        op=mybir.AluOpType.add)
                ht.append(h_sb)

            # E_reg^T = pred_W^T h^T + pred_b + E_new^T[:, :REGPAD]
            for s0 in range(0, REGPAD, NSTRIP):
                pe = pp1.tile([P, NSTRIP], f32, tag="misc")
                nc.tensor.matmul(pe[:], lhsT=pw_sb[:, 0:EDIM],
                                 rhs=ht[0][:, s0:s0 + NSTRIP],
                                 start=True, stop=False)
                nc.tensor.matmul(pe[:], lhsT=pw_sb[:, EDIM:2 * EDIM],
                                 rhs=ht[1][:, s0:s0 + NSTRIP],
                                 start=False, stop=True)
                er = sp.tile([P, NSTRIP], f32, tag="er")
                nc.scalar.activation(
                    out=er[:], in_=pe[:],
                    func=mybir.ActivationFunctionType.Identity,
                    bias=pb_sb[:, 0:1], scale=1.0)
                nc.vector.tensor_tensor(out=er[:], in0=er[:],
                                        in1=ent_sb[:, s0:s0 + NSTRIP],
                                        op=mybir.AluOpType.add)
                nc.sync.dma_start(out=ereg_out.ap()[:, s0:s0 + NSTRIP],
                                  in_=er[:])
    nc.compile()
    return nc


# --------------------------------------------------------------------------
# host driver
# --------------------------------------------------------------------------

def _run(nc, in_maps, trace=False):
    return bass_utils.run_bass_kernel_spmd(
        nc, in_maps, core_ids=list(range(NCORE)), trace=trace)


def kernel(E_weight, R_weight, rgcn_W, rgcn_Wroot, rgcn_b,
           mp0_W, mp0_Wroot, mp0_b, mp1_W, mp1_Wroot, mp1_b,
           sa_w1, sa_b1, sa_w2, pred_W, pred_b,
           h_idx, r_idx, edge_index, edge_type,
           mp0_edge_index, mp0_edge_type, mp0_eids,
           mp1_edge_index, mp1_edge_type, mp1_eids,
           _trace=False):
    E_weight = np.asarray(E_weight, np.float32)
    R_weight = np.asarray(R_weight, np.float32)
    rgcn_W = np.asarray(rgcn_W, np.float32)
    rgcn_Wroot = np.asarray(rgcn_Wroot, np.float32)
    rgcn_b = np.asarray(rgcn_b, np.float32)
    sa_w1 = np.asarray(sa_w1, np.float32)
    sa_b1 = np.asarray(sa_b1, np.float32)
    sa_w2 = np.asarray(sa_w2, np.float32)
    pred_W = np.asarray(pred_W, np.float32)
    pred_b = np.asarray(pred_b, np.float32)
    h_idx = np.asarray(h_idx).astype(np.int64)
    r_idx = np.asarray(r_idx).astype(np.int64)
    ei = np.asarray(edge_index).astype(np.int64)
    et = np.asarray(edge_type).astype(np.int64)
    mps_int = []
    for mei, met, meid in ((mp0_edge_index, mp0_edge_type, mp0_eids),
                           (mp1_edge_index, mp1_edge_type, mp1_eids)):
        mei = np.asarray(mei).astype(np.int64)
        mps_int.append((mei[0], mei[1], np.asarray(met).astype(np.int64),
                        np.asarray(meid).astype(np.int64)))

    key = "programs"
    if key not in _cache:
        plan, nc1, srcix, dstl = _plan_k1(ei[0], ei[1], et)
        plan2, nc2, dstl2, gidx, root_gidx = _plan_k2(mps_int)
        prog1 = _build_k1(plan, nc1)
        prog2 = _build_k2(plan2, nc2, len(gidx))
        _cache[key] = (prog1, prog2, nc1, srcix, dstl, nc2, dstl2, gidx,
                       root_gidx)
    prog1, prog2, nc1, srcix, dstl, nc2, dstl2, gidx, root_gidx = _cache[key]

    iota = _iota_tile()
    w_all = rgcn_W.transpose(1, 0, 2).reshape(EDIM, NR * EDIM).copy()
    in_maps1 = []
    for c in range(NCORE):
        in_maps1.append({
            "e_full": E_weight,
            "e_t": np.ascontiguousarray(E_weight[c * SH:(c + 1) * SH].T),
            "w_all": w_all,
            "wroot": rgcn_Wroot,
            "bvec": rgcn_b[:, None].copy(),
            "iota": iota,
            "srcidx": srcix[c],
            "dstl": dstl[c],
        })
    res1 = _run(prog1, in_maps1, trace=_trace)
    E_new = np.concatenate([np.asarray(r["e_newt_out"]).T
                            for r in res1.results], axis=0)

    # host exchange (stand-in for all-to-all): gather rows needed by launch 2
    xht = np.ascontiguousarray(E_new[h_idx].T)
    rht = np.ascontiguousarray(R_weight[r_idx].T)
    mpx = E_new[gidx] if len(gidx) else np.zeros((1, EDIM), np.float32)
    mproott = np.zeros((EDIM, 2 * REGPAD), np.float32)
    for m in range(2):
        mproott[:, m * REGPAD:m * REGPAD + NREG] = E_new[root_gidx[m]].T
    mpw = np.zeros((EDIM, 2 * NMP_REL * HID), np.float32)
    for m, W in enumerate((mp0_W, mp1_W)):
        W = np.asarray(W, np.float32)
        for r in range(NMP_REL):
            mpw[:, (m * NMP_REL + r) * HID:(m * NMP_REL + r + 1) * HID] = W[r]
    mpwroot = np.zeros((EDIM, 2 * HID), np.float32)
    mpwroot[:, 0:HID] = np.asarray(mp0_Wroot, np.float32)
    mpwroot[:, HID:2 * HID] = np.asarray(mp1_Wroot, np.float32)
    mpb = np.zeros((P, 4), np.float32)
    mpb[:, 0] = np.asarray(mp0_b, np.float32)[:P]
    mpb[:, 1] = np.asarray(mp0_b, np.float32)[P:]
    mpb[:, 2] = np.asarray(mp1_b, np.float32)[:P]
    mpb[:, 3] = np.asarray(mp1_b, np.float32)[P:]
    saw1 = np.zeros((P, 2 * ATT), np.float32)
    saw1[:, 0:ATT] = sa_w1[0:P]        # k-tile 0 (HID rows 0..127)
    saw1[:, ATT:2 * ATT] = sa_w1[P:]   # k-tile 1 (HID rows 128..255)
    predw = np.zeros((P, 2 * EDIM), np.float32)
    predw[:, 0:EDIM] = pred_W[0:P]
    predw[:, EDIM:] = pred_W[P:]

    in_maps2 = []
    for c in range(NCORE):
        in_maps2.append({
            "ent": np.ascontiguousarray(E_new[c * SH:(c + 1) * SH].T),
            "xht": xht, "rht": rht,
            "mpx": mpx, "mpdstl": dstl2, "mproott": mproott,
            "mpw": mpw, "mpwroot": mpwroot, "mpb": mpb,
            "saw1": saw1, "sab1": sa_b1[:, None].copy(),
            "saw2": sa_w2.copy(), "predw": predw,
            "predb": pred_b[:, None].copy(),
            "ones_row": np.ones((1, P), np.float32),
            "iota": iota,
        })
    kernel._last = dict(prog1=prog1, in_maps1=in_maps1, prog2=prog2,
                        in_maps2=in_maps2)
    res2 = _run(prog2, in_maps2, trace=_trace)
    pred = np.concatenate([np.asarray(r["pred_out"]) for r in res2.results],
                          axis=1)
    E_reg = np.asarray(res2.results[0]["ereg_out"])[:, :NREG].T.copy()
    if _trace:
        kernel._last_times = (res1.exec_time_ns, res2.exec_time_ns)
    return E_reg, pred


# --------------------------------------------------------------------------
# benchmarking helpers (not used by the grading path)
# --------------------------------------------------------------------------

def _timed_bench(nc, in_maps, iters=8, inner=1):
    import time as _time
    import jax
    from jax.sharding import Mesh, NamedSharding, PartitionSpec
    from jax.experimental.shard_map import shard_map
    from concourse import bass2jax
    bass2jax.install_neuronx_cc_hook()
    pname = (nc.partition_id_tensor.name if nc.partition_id_tensor
             else None)
    in_names, out_names, out_avals, zero_outs = [], [], [], []
    for alloc in nc.m.functions[0].allocations:
        if not isinstance(alloc, mybir.MemoryLocationSet):
            continue
        name = alloc.memorylocations[0].name
        if alloc.kind == "ExternalInput":
            if name != pname:
                in_names.append(name)
        elif alloc.kind == "ExternalOutput":
            out_names.append(name)
            out_avals.append(jax.core.ShapedArray(tuple(alloc.tensor_shape),
                                                  dt.np(alloc.dtype)))
            zero_outs.append(np.zeros(tuple(alloc.tensor_shape),
                                      dt.np(alloc.dtype)))
    n_params = len(in_names)
    n_outs = len(out_names)
    all_in = tuple(in_names + out_names
                   + ([pname] if pname is not None else []))

    def _body(*args):
        operands = list(args)
        if pname is not None:
            operands.append(bass2jax.partition_id_tensor())
        for _ in range(inner):
            outs = bass2jax._bass_exec_p.bind(
                *operands, out_avals=tuple(out_avals), in_names=all_in,
                out_names=tuple(out_names), lowering_input_output_aliases=(),
                sim_require_finite=True, sim_require_nnan=True, nc=nc)
        return tuple(outs)

    devices = jax.devices()[:NCORE]
    mesh = Mesh(np.asarray(devices), ("core",))
    fn = jax.jit(
        shard_map(_body, mesh=mesh,
                  in_specs=(PartitionSpec("core"),) * (n_params + n_outs),
                  out_specs=(PartitionSpec("core"),) * n_outs,
                  check_rep=False),
        keep_unused=True)
    sh = NamedSharding(mesh, PartitionSpec("core"))
    dev_in = [jax.device_put(
        np.concatenate([np.asarray(m[name]) for m in in_maps], axis=0), sh)
        for name in in_names]
    dev_zero = [jax.device_put(
        np.concatenate([z] * NCORE, axis=0), sh) for z in zero_outs]
    out = fn(*dev_in, *dev_zero)
    jax.block_until_ready(out)
    ts = []
    for _ in range(iters):
        t0 = _time.perf_counter()
        out = fn(*dev_in, *dev_zero)
        jax.block_until_ready(out)
        ts.append(_time.perf_counter() - t0)
    return ts


def _build_noop():
    nc = bacc.Bacc("TRN2", target_bir_lowering=False, debug=False,
                   num_devices=NCORE)
    a = nc.dram_tensor("a", [P, P], f32, kind="ExternalInput")
    o = nc.dram_tensor("o", [P, P], f32, kind="ExternalOutput")
    with tile.TileContext(nc) as tc:
        with tc.tile_pool(name="sb", bufs=1) as sp:
            t = sp.tile([P, P], f32, tag="t")
            nc.sync.dma_start(out=t[:], in_=a.ap()[:, :])
            nc.sync.dma_start(out=o.ap()[:, :], in_=t[:])
    nc.compile()
    return nc
